# revision 10
# baseline (speedup 1.0000x reference)
"""Bidirectional GQA attention block (B=4,T=2048,C=2048,H=16,KVH=4) on 8 TRN2 cores.

Sharding: data-parallel over (batch, seq-half): core c handles batch b=c//2 and
query tokens [r0, r0+1024).  Each core projects q/k/v ONLY for its own 1024
tokens; the (roped) k and v halves are exchanged between the two cores of a
batch via a paired AllGather, so no projection work is duplicated.  All
k/v/rope-table token indexing is in original order (rank order == token order),
which keeps the SPMD program identical across cores.

v4 (bf16 + kv AllGather): all matmul operands bf16 with fp32 PSUM accumulation.
 - x input is only the core's own half (C x 1024).
 - softmax denominator: bf16 pair-tree on DVE + a short ones-matmul.
 - k roped before the exchange (per-token rope commutes with the gather).
 - rmsnorm sqrt batched; scales folded into rope tables / ACT copies.
 - y^T written into the dead q^T tiles; wo prefetched during attention.
"""
import sys
import os

sys.path.insert(0, "/opt/trn_rl_repo")

import numpy as np
import ml_dtypes

BF = ml_dtypes.bfloat16

B, T, C = 4, 2048, 2048
N_HEAD, N_KV_HEAD = 16, 4
HEAD_DIM = C // N_HEAD  # 128
KV_DIM = N_KV_HEAD * HEAD_DIM  # 512
EPS = 1e-5
TQ = 1024  # query tokens per core
N_CORES = 8

_CACHE = {}


def _build_nc(reps=1, trace_sim=False):
    import concourse.bass as bass
    import concourse.mybir as mybir
    import concourse.tile as tile
    from concourse import bacc

    F32 = mybir.dt.float32
    BF16 = mybir.dt.bfloat16
    AF = mybir.ActivationFunctionType
    ALU = mybir.AluOpType
    RG = [[0, 1], [2, 3], [4, 5], [6, 7]]

    nc = bacc.Bacc("TRN2", target_bir_lowering=False, debug=False)

    def ein(name, shape, dt=BF16):
        return nc.dram_tensor(name, shape, dt, kind="ExternalInput").ap()

    xT = ein("xT", [C, TQ])         # x[b].T own-token half (c_in, tok)
    wq = ein("wq", [C, C])
    wk = ein("wk", [C, KV_DIM])
    wv = ein("wv", [C, KV_DIM])
    wo = ein("wo", [C, C])
    c2q = ein("c2q", [128, TQ])     # [cos;cos] / sqrt(HEAD_DIM), own-token slice
    s2q = ein("s2q", [128, TQ])     # [sin;-sin] / sqrt(HEAD_DIM)
    c2k = ein("c2k", [128, TQ])     # own-token slice, unscaled
    s2k = ein("s2k", [128, TQ])
    qnw = ein("qnw", [128, 16], F32)  # q_norm_w.reshape(16,128).T
    knw = ein("knw", [128, 4], F32)
    out = nc.dram_tensor("out", [TQ, C], F32, kind="ExternalOutput").ap()

    ones_d = nc.inline_tensor(np.ones((128, 1), BF), name="onesc").ap()
    onesq_d = nc.inline_tensor(np.full((128, 1), 1.0 / C, BF), name="onesqc").ap()
    onesk_d = nc.inline_tensor(np.full((128, 1), 1.0 / KV_DIM, BF), name="oneskc").ap()
    eps_d = nc.inline_tensor(np.full((1, 1), EPS, np.float32), name="epsc").ap()

    def rp(ap, p=128):
        # (c*p, n) -> (p, c, n)
        return ap.rearrange("(c p) n -> p c n", p=p)

    with tile.TileContext(nc, trace_sim=trace_sim) as tc:
        with tc.tile_pool(name="const", bufs=1) as cpool:
            ones_t = cpool.tile([128, 1], BF16, name="ones_t")
            nc.sync.dma_start(ones_t[:], ones_d)
            onesq_t = cpool.tile([128, 1], BF16, name="onesq_t")
            nc.sync.dma_start(onesq_t[:], onesq_d)
            onesk_t = cpool.tile([128, 1], BF16, name="onesk_t")
            nc.sync.dma_start(onesk_t[:], onesk_d)
            eps_t = cpool.tile([1, 1], F32, name="eps_t")
            nc.sync.dma_start(eps_t[:], eps_d)
            qnw_t = cpool.tile([128, 16], F32, name="qnw_t")
            nc.sync.dma_start(qnw_t[:], qnw)
            knw_t = cpool.tile([128, 4], F32, name="knw_t")
            nc.sync.dma_start(knw_t[:], knw)

            for rep in range(reps):
                rr = rep % 2
                # persistent activations for this rep
                pact = tc.alloc_tile_pool(name="acts", bufs=1)
                kTt = [pact.tile([128, T], BF16, name=f"kT{g}", tag=f"kT{g}_{rr}")
                       for g in range(N_KV_HEAD)]
                vts = [pact.tile([128, KV_DIM], BF16, name=f"vt{i}",
                                 tag=f"vt{i}_{rr}") for i in range(16)]
                qTt = [pact.tile([128, TQ], BF16, name=f"qT{h}", tag=f"qT{h}_{rr}")
                       for h in range(N_HEAD)]
                yTt = qTt  # y^T reuses the dead roped-q tiles
                kOwn = [pact.tile([128, TQ], BF16, name=f"ko{g}", tag=f"ko{g}_{rr}")
                        for g in range(N_KV_HEAD)]
                vOwn = [pact.tile([128, KV_DIM], BF16, name=f"vo{i}",
                                  tag=f"vo{i}_{rr}") for i in range(8)]

                # rope tables; swap scratch
                ptab = tc.alloc_tile_pool(name="tabs", bufs=1)
                pswk = tc.alloc_tile_pool(name="swk", bufs=2)
                pdram = tc.alloc_tile_pool(name="dramp", bufs=1, space="DRAM")
                kv_in = pdram.tile([TQ, TQ], BF16, name="kv_in", tag=f"kvi{rr}")
                kv_out = pdram.tile([2 * TQ, TQ], BF16, name="kv_out",
                                    tag=f"kvo{rr}")
                # v region views (token-major (1024, 512))
                vin_v = kv_in[512:1024, :].rearrange("a (b c) -> (a b) c", b=2)
                vout_v0 = kv_out[512:1024, :].rearrange("a (b c) -> (a b) c", b=2)
                vout_v1 = kv_out[1536:2048, :].rearrange("a (b c) -> (a b) c", b=2)

                # q rope swap pool (hoisted: lets the h=0 swap DMA prefetch)
                pqs = tc.alloc_tile_pool(name="qsw", bufs=3)
                # rmsnorm stats
                pst = tc.alloc_tile_pool(name="stats", bufs=1)
                ssqk_sb = pst.tile([1, TQ], F32, name="ssqk_sb", tag=f"ssqk{rr}")
                ssqq_sb = pst.tile([1, TQ], F32, name="ssqq_sb", tag=f"ssqq{rr}")
                rs_q = pst.tile([1, TQ], F32, name="rs_q", tag=f"rsq{rr}")
                rs_k = pst.tile([1, TQ], F32, name="rs_k", tag=f"rsk{rr}")
                # x tiles for own tokens (kv proj and q proj)
                pxq = tc.alloc_tile_pool(name="xqp", bufs=1)
                xqs = [pxq.tile([128, TQ], BF16, name=f"xq{kc}", tag=f"xq{kc}_{rr}")
                       for kc in range(16)]

                # ---------------- P1a: k^T and v projections (own half) -------
                pwq = tc.alloc_tile_pool(name="wqlp", bufs=3)
                pwkv = tc.alloc_tile_pool(name="wkv", bufs=1)
                wk4 = [pwkv.tile([128, 4, KV_DIM], BF16, name=f"wk4_{i}",
                                 tag=f"wk4_{i}_{rr}") for i in range(4)]
                wv4 = [pwkv.tile([128, 4, KV_DIM], BF16, name=f"wv4_{i}",
                                 tag=f"wv4_{i}_{rr}") for i in range(4)]
                nc.sync.dma_start(wk4[0][:], rp(wk)[:, 0:4, :])
                for kc in range(4):
                    nc.sync.dma_start(xqs[kc][:], rp(xT)[:, kc, :])
                for i in range(1, 4):
                    nc.sync.dma_start(wk4[i][:], rp(wk)[:, 4 * i:4 * i + 4, :])
                for kc in range(4, 16):
                    nc.sync.dma_start(xqs[kc][:], rp(xT)[:, kc, :])
                for i in range(4):
                    nc.sync.dma_start(wv4[i][:], rp(wv)[:, 4 * i:4 * i + 4, :])
                c2ks = ptab.tile([128, TQ], BF16, name="c2ks", tag=f"c2ks{rr}")
                nc.sync.dma_start(c2ks[:], c2k)
                s2ks = ptab.tile([128, TQ], BF16, name="s2ks", tag=f"s2ks{rr}")
                nc.sync.dma_start(s2ks[:], s2k)
                c2qs = ptab.tile([128, TQ], BF16, name="c2qs", tag=f"c2qs{rr}")
                nc.sync.dma_start(c2qs[:], c2q)
                s2qs = ptab.tile([128, TQ], BF16, name="s2qs", tag=f"s2qs{rr}")
                nc.sync.dma_start(s2qs[:], s2q)

                with tc.tile_pool(name="pp1", bufs=3, space="PSUM") as pp, \
                     tc.tile_pool(name="sq1", bufs=3) as pt, \
                     tc.tile_pool(name="ssqkp", bufs=2, space="PSUM") as pps:
                    for tk in range(2):
                        ssqk_ps = pps.tile([1, 512], F32, name="ssqk", tag="ssqk")
                        for co in range(4):
                            ps = pp.tile([128, 512], F32, name="psk", tag="ps")
                            for kc in range(16):
                                nc.tensor.matmul(
                                    ps[:],
                                    wk4[kc // 4][:, kc % 4, co * 128:(co + 1) * 128],
                                    xqs[kc][:, tk * 512:(tk + 1) * 512],
                                    start=(kc == 0),
                                    stop=(kc == 15),
                                )
                            nc.scalar.activation(
                                kOwn[co][:, tk * 512:(tk + 1) * 512], ps[:],
                                AF.Copy, scale=knw_t[:, co:co + 1])
                            sq = pt.tile([128, 512], BF16, name="sqk", tag="sq")
                            nc.scalar.activation(sq[:], ps[:], AF.Square)
                            nc.tensor.matmul(
                                ssqk_ps[:], onesk_t[:], sq[:],
                                start=(co == 0), stop=(co == 3))
                        nc.scalar.activation(
                            ssqk_sb[:, tk * 512:(tk + 1) * 512], ssqk_ps[:], AF.Copy)
                        for vt in range(4):
                            ps = pp.tile([128, 512], F32, name="psv", tag="ps")
                            for kc in range(16):
                                nc.tensor.matmul(
                                    ps[:],
                                    xqs[kc][:, tk * 512 + vt * 128:
                                            tk * 512 + (vt + 1) * 128],
                                    wv4[kc // 4][:, kc % 4, :],
                                    start=(kc == 0),
                                    stop=(kc == 15),
                                )
                            nc.vector.tensor_copy(vOwn[tk * 4 + vt][:], ps[:])
                pwkv.release()

                # ------- k-side: rmsnorm, rope own half, kv AllGather ----------
                with tc.tile_pool(name="kprep", bufs=1) as pkp:
                    sdk = pkp.tile([1, TQ], F32, name="sdk")
                    nc.scalar.activation(sdk[:], ssqk_sb[:], AF.Sqrt, bias=eps_t[:])
                    nc.vector.reciprocal(rs_k[:], sdk[:])
                    bck = pkp.tile([128, TQ], F32, name="bck")
                    nc.gpsimd.partition_broadcast(bck[:], rs_k[:])
                    nc.vector.tensor_mul(c2ks[:], c2ks[:], bck[:])
                    nc.vector.tensor_mul(s2ks[:], s2ks[:], bck[:])
                    for g in range(N_KV_HEAD):
                        ksw = pswk.tile([128, TQ], BF16, name="ksw", tag="ksw")
                        nc.sync.dma_start(ksw[0:64, :], kOwn[g][64:128, :])
                        nc.sync.dma_start(ksw[64:128, :], kOwn[g][0:64, :])
                        nc.vector.tensor_mul(kOwn[g][:], kOwn[g][:], c2ks[:])
                        nc.vector.tensor_mul(ksw[:], ksw[:], s2ks[:])
                        nc.vector.tensor_add(kOwn[g][:], kOwn[g][:], ksw[:])
                        nc.sync.dma_start(kv_in[g * 128:(g + 1) * 128, :],
                                          kOwn[g][:])
                    for i in range(8):
                        nc.sync.dma_start(vin_v[i * 128:(i + 1) * 128, :],
                                          vOwn[i][:])
                    nc.gpsimd.collective_compute(
                        "AllGather",
                        ALU.bypass,
                        replica_groups=RG,
                        ins=[kv_in.opt()],
                        outs=[kv_out.opt()],
                    )
                    # read back full k (roped) and v in original token order
                    for g in range(N_KV_HEAD):
                        nc.sync.dma_start(kTt[g][:, 0:TQ],
                                          kv_out[g * 128:(g + 1) * 128, :])
                        nc.sync.dma_start(
                            kTt[g][:, TQ:T],
                            kv_out[TQ + g * 128:TQ + (g + 1) * 128, :])
                    for i in range(8):
                        nc.sync.dma_start(vts[i][:],
                                          vout_v0[i * 128:(i + 1) * 128, :])
                        nc.sync.dma_start(vts[8 + i][:],
                                          vout_v1[i * 128:(i + 1) * 128, :])

                # ---------------- P1b: q^T projection --------
                pw = pwq
                with tc.tile_pool(name="sq2", bufs=3) as pt, \
                     tc.tile_pool(name="pp2", bufs=3, space="PSUM") as pp, \
                     tc.tile_pool(name="ssqq0", bufs=1, space="PSUM") as pps0, \
                     tc.tile_pool(name="ssqq1", bufs=1, space="PSUM") as pps1:
                    ssq_ps = [pps0.tile([1, 512], F32, name="ssqq0", tag="ssqq0"),
                              pps1.tile([1, 512], F32, name="ssqq1", tag="ssqq1")]
                    for h in range(16):
                        wql = pw.tile([128, 16, 128], BF16, name="wql", tag="wql")
                        nc.sync.dma_start(wql[:], rp(wq)[:, :, h * 128:(h + 1) * 128])
                        for tq in range(2):
                            ps = pp.tile([128, 512], F32, name="psq", tag="ps")
                            for kc in range(16):
                                nc.tensor.matmul(
                                    ps[:],
                                    wql[:, kc, :],
                                    xqs[kc][:, tq * 512:(tq + 1) * 512],
                                    start=(kc == 0),
                                    stop=(kc == 15),
                                )
                            nc.scalar.activation(
                                qTt[h][:, tq * 512:(tq + 1) * 512], ps[:],
                                AF.Copy, scale=qnw_t[:, h:h + 1])
                            sq = pt.tile([128, 512], BF16, name="sqq", tag="sq")
                            nc.scalar.activation(sq[:], ps[:], AF.Square)
                            nc.tensor.matmul(
                                ssq_ps[tq][:], onesq_t[:], sq[:],
                                start=(h == 0), stop=(h == 15))
                    for tq in range(2):
                        nc.scalar.activation(
                            ssqq_sb[:, tq * 512:(tq + 1) * 512], ssq_ps[tq][:],
                            AF.Copy)
                pwq.release()
                pxq.release()

                # ------------- q-side rmsnorm scales -> rope tables -----------
                with tc.tile_pool(name="bcp", bufs=1) as pbc:
                    sdq = pbc.tile([1, TQ], F32, name="sdq")
                    nc.scalar.activation(sdq[:], ssqq_sb[:], AF.Sqrt, bias=eps_t[:])
                    nc.vector.reciprocal(rs_q[:], sdq[:])
                    bcq = pbc.tile([128, TQ], F32, name="bcq")
                    nc.gpsimd.partition_broadcast(bcq[:], rs_q[:])
                    nc.vector.tensor_mul(c2qs[:], c2qs[:], bcq[:])
                    nc.vector.tensor_mul(s2qs[:], s2qs[:], bcq[:])
                pst.release()

                # ---------------- P2: attention (+ P3 wo prefetch) -------------
                pwo = tc.alloc_tile_pool(name="wop", bufs=2)
                with tc.tile_pool(name="Sp", bufs=2) as pS, \
                     tc.tile_pool(name="dt8", bufs=2) as pd8, \
                     tc.tile_pool(name="dt4", bufs=2) as pd4, \
                     tc.tile_pool(name="yev", bufs=3) as pye, \
                     tc.tile_pool(name="sps", bufs=2, space="PSUM") as ppS, \
                     tc.tile_pool(name="denp", bufs=2, space="PSUM") as ppd, \
                     tc.tile_pool(name="ytp", bufs=2, space="PSUM") as ppy:
                    for h in range(N_HEAD):
                        g = h // 4
                        # rope q in place
                        qsw = pqs.tile([128, TQ], BF16, name="qsw", tag="qsw")
                        nc.sync.dma_start(qsw[0:64, :], qTt[h][64:128, :])
                        nc.sync.dma_start(qsw[64:128, :], qTt[h][0:64, :])
                        nc.vector.tensor_mul(qTt[h][:], qTt[h][:], c2qs[:])
                        nc.vector.tensor_mul(qsw[:], qsw[:], s2qs[:])
                        nc.vector.tensor_add(qTt[h][:], qTt[h][:], qsw[:])
                        for qc in range(2):
                            qsl = qTt[h][:, qc * 512:(qc + 1) * 512]
                            S_sb = pS.tile([128, 16, 512], BF16, name="S_sb", tag="S")
                            for j in range(8):
                                sps = ppS.tile([128, 2, 512], F32, name="sps",
                                               tag="sps")
                                for i in range(2):
                                    kc = 2 * j + i
                                    nc.tensor.matmul(
                                        sps[:, i, :],
                                        kTt[g][:, kc * 128:(kc + 1) * 128],
                                        qsl,
                                        start=True,
                                        stop=True,
                                    )
                                nc.scalar.activation(
                                    S_sb[:, 2 * j:2 * j + 2, :], sps[:], AF.Exp)
                            # denominator: bf16 pair-tree 16 -> 4 on DVE
                            t8 = pd8.tile([128, 8, 512], BF16, name="t8", tag="t8")
                            for i in range(8):
                                nc.vector.tensor_add(
                                    t8[:, i, :], S_sb[:, 2 * i, :],
                                    S_sb[:, 2 * i + 1, :])
                            t4 = pd4.tile([128, 4, 512], BF16, name="t4", tag="t4")
                            for i in range(4):
                                nc.vector.tensor_add(
                                    t4[:, i, :], t8[:, 2 * i, :],
                                    t8[:, 2 * i + 1, :])
                            # y^T = v^T S accumulated over 16 token chunks
                            yt_ps = ppy.tile([128, 512], F32, name="ytp", tag="ytp")
                            for kc in range(16):
                                nc.tensor.matmul(
                                    yt_ps[:],
                                    vts[kc][:, g * 128:(g + 1) * 128],
                                    S_sb[:, kc, :],
                                    start=(kc == 0),
                                    stop=(kc == 15),
                                )
                            den_ps = ppd.tile([1, 512], F32, name="den", tag="den")
                            for i in range(4):
                                nc.tensor.matmul(
                                    den_ps[:], ones_t[:], t4[:, i, :],
                                    start=(i == 0), stop=(i == 3))
                            rcp = pye.tile([1, 512], F32, name="rcp", tag="rcp")
                            nc.vector.reciprocal(rcp[:], den_ps[:])
                            bcr = pye.tile([128, 512], F32, name="bcr", tag="bcr")
                            nc.gpsimd.partition_broadcast(bcr[:], rcp[:])
                            nc.vector.tensor_mul(
                                yTt[h][:, qc * 512:(qc + 1) * 512], yt_ps[:], bcr[:])

                # ---------------- P3: output projection ----------------
                with tc.tile_pool(name="ev3", bufs=4) as pe3, \
                     tc.tile_pool(name="pp3", bufs=4, space="PSUM") as pp3:
                    for co in range(4):
                        woc = pwo.tile([128, 16, 512], BF16, name="woc",
                                       tag="woc")
                        for yc in range(16):
                            nc.sync.dma_start(
                                woc[:, yc, :],
                                rp(wo)[:, yc, co * 512:(co + 1) * 512])
                        for qt in range(8):
                            ps = pp3.tile([128, 512], F32, name="pso", tag="ps")
                            for yc in range(16):
                                nc.tensor.matmul(
                                    ps[:],
                                    yTt[yc][:, qt * 128:(qt + 1) * 128],
                                    woc[:, yc, :],
                                    start=(yc == 0),
                                    stop=(yc == 15),
                                )
                            osb = pe3.tile([128, 512], F32, name="osb",
                                           tag="osb")
                            nc.vector.tensor_copy(osb[:], ps[:])
                            nc.sync.dma_start(
                                out[qt * 128:(qt + 1) * 128,
                                    co * 512:(co + 1) * 512],
                                osb[:],
                            )
                pwo.release()
                pqs.release()
                pdram.release()
                pswk.release()
                ptab.release()
                pact.release()

    nc.compile()
    return nc


def _make_in_maps(inputs):
    x = np.asarray(inputs["x"], np.float32)
    cos = np.asarray(inputs["cos"], np.float32)
    sin = np.asarray(inputs["sin"], np.float32)
    wq = np.ascontiguousarray(np.asarray(inputs["wq"], np.float32)).astype(BF)
    wk = np.ascontiguousarray(np.asarray(inputs["wk"], np.float32)).astype(BF)
    wv = np.ascontiguousarray(np.asarray(inputs["wv"], np.float32)).astype(BF)
    wo = np.ascontiguousarray(np.asarray(inputs["wo"], np.float32)).astype(BF)
    qnw = np.ascontiguousarray(
        np.asarray(inputs["q_norm_w"], np.float32).reshape(16, 128).T)
    knw = np.ascontiguousarray(
        np.asarray(inputs["k_norm_w"], np.float32).reshape(4, 128).T)

    cf = cos[0, :, 0, :].T  # (64, T)
    sf = sin[0, :, 0, :].T
    c2 = np.concatenate([cf, cf], 0)  # (128, T)
    s2 = np.concatenate([sf, -sf], 0)
    scale = 1.0 / np.sqrt(np.float32(HEAD_DIM))

    in_maps = []
    for c in range(N_CORES):
        b, r0 = c // 2, (c % 2) * TQ
        xTb = x[b].T  # (C, T)
        in_maps.append({
            "xT": np.ascontiguousarray(xTb[:, r0:r0 + TQ]).astype(BF),
            "wq": wq, "wk": wk, "wv": wv, "wo": wo,
            "c2q": np.ascontiguousarray(c2[:, r0:r0 + TQ] * scale).astype(BF),
            "s2q": np.ascontiguousarray(s2[:, r0:r0 + TQ] * scale).astype(BF),
            "c2k": np.ascontiguousarray(c2[:, r0:r0 + TQ]).astype(BF),
            "s2k": np.ascontiguousarray(s2[:, r0:r0 + TQ]).astype(BF),
            "qnw": qnw, "knw": knw,
        })
    return in_maps


def run(inputs, **spmd_kwargs):
    from concourse import bass_utils

    if "nc" not in _CACHE:
        _CACHE["nc"] = _build_nc()
    nc = _CACHE["nc"]
    res = bass_utils.run_bass_kernel_spmd(
        nc, _make_in_maps(inputs), core_ids=list(range(N_CORES)), **spmd_kwargs
    )
    out = np.empty((B, T, C), np.float32)
    for c in range(N_CORES):
        b, r0 = c // 2, (c % 2) * TQ
        out[b, r0:r0 + TQ, :] = res.results[c]["out"]
    return out, res


def kernel(**inputs):
    out, _ = run(inputs)
    return out


# revision 12
# speedup vs baseline: 1.2387x; 1.2387x over previous
"""Bidirectional GQA attention block (B=4,T=2048,C=2048,H=16,KVH=4) on 8 TRN2 cores.

Sharding: data-parallel over (batch, seq-half): core c handles batch b=c//2 and
query tokens [r0, r0+1024).  Each core projects q/k/v ONLY for its own 1024
tokens; the (roped) k and v halves are exchanged between the two cores of a
batch via a paired AllGather, so no projection work is duplicated.  All
k/v/rope-table token indexing is in original order (rank order == token order),
which keeps the SPMD program identical across cores.

v4 (bf16 + kv AllGather): all matmul operands bf16 with fp32 PSUM accumulation.
 - x input is only the core's own half (C x 1024).
 - softmax denominator: bf16 pair-tree on DVE + a short ones-matmul.
 - k roped before the exchange (per-token rope commutes with the gather).
 - rmsnorm sqrt batched; scales folded into rope tables / ACT copies.
 - y^T written into the dead q^T tiles; wo prefetched during attention.
"""
import sys
import os

sys.path.insert(0, "/opt/trn_rl_repo")

import numpy as np
import ml_dtypes

BF = ml_dtypes.bfloat16

B, T, C = 4, 2048, 2048
N_HEAD, N_KV_HEAD = 16, 4
HEAD_DIM = C // N_HEAD  # 128
KV_DIM = N_KV_HEAD * HEAD_DIM  # 512
EPS = 1e-5
TQ = 1024  # query tokens per core
N_CORES = 8

_CACHE = {}


def _build_nc(reps=1, trace_sim=False):
    import concourse.bass as bass
    import concourse.mybir as mybir
    import concourse.tile as tile
    from concourse import bacc

    F32 = mybir.dt.float32
    BF16 = mybir.dt.bfloat16
    AF = mybir.ActivationFunctionType
    ALU = mybir.AluOpType
    RG = [[0, 1], [2, 3], [4, 5], [6, 7]]

    nc = bacc.Bacc("TRN2", target_bir_lowering=False, debug=False)

    def ein(name, shape, dt=BF16):
        return nc.dram_tensor(name, shape, dt, kind="ExternalInput").ap()

    xT = ein("xT", [C, TQ])         # x[b].T own-token half (c_in, tok)
    wq = ein("wq", [C, C])
    wk = ein("wk", [C, KV_DIM])
    wv = ein("wv", [C, KV_DIM])
    wo = ein("wo", [C, C])
    c2q = ein("c2q", [128, TQ])     # [cos;cos] / sqrt(HEAD_DIM), own-token slice
    s2q = ein("s2q", [128, TQ])     # [sin;-sin] / sqrt(HEAD_DIM)
    c2k = ein("c2k", [128, TQ])     # own-token slice, unscaled
    s2k = ein("s2k", [128, TQ])
    qnw = ein("qnw", [128, 16], F32)  # q_norm_w.reshape(16,128).T
    knw = ein("knw", [128, 4], F32)
    out = nc.dram_tensor("out", [TQ, C], F32, kind="ExternalOutput").ap()

    ones_d = nc.inline_tensor(np.ones((128, 1), BF), name="onesc").ap()
    onesq_d = nc.inline_tensor(np.full((128, 1), 1.0 / C, BF), name="onesqc").ap()
    onesk_d = nc.inline_tensor(np.full((128, 1), 1.0 / KV_DIM, BF), name="oneskc").ap()
    eps_d = nc.inline_tensor(np.full((1, 1), EPS, np.float32), name="epsc").ap()

    def rp(ap, p=128):
        # (c*p, n) -> (p, c, n)
        return ap.rearrange("(c p) n -> p c n", p=p)

    with tile.TileContext(nc, trace_sim=trace_sim) as tc:
        with tc.tile_pool(name="const", bufs=1) as cpool:
            ones_t = cpool.tile([128, 1], BF16, name="ones_t")
            nc.sync.dma_start(ones_t[:], ones_d)
            onesq_t = cpool.tile([128, 1], BF16, name="onesq_t")
            nc.sync.dma_start(onesq_t[:], onesq_d)
            onesk_t = cpool.tile([128, 1], BF16, name="onesk_t")
            nc.sync.dma_start(onesk_t[:], onesk_d)
            eps_t = cpool.tile([1, 1], F32, name="eps_t")
            nc.sync.dma_start(eps_t[:], eps_d)
            qnw_t = cpool.tile([128, 16], F32, name="qnw_t")
            nc.sync.dma_start(qnw_t[:], qnw)
            knw_t = cpool.tile([128, 4], F32, name="knw_t")
            nc.sync.dma_start(knw_t[:], knw)

            for rep in range(reps):
                rr = rep % 2
                # persistent activations for this rep
                pact = tc.alloc_tile_pool(name="acts", bufs=1)
                kTt = [pact.tile([128, T], BF16, name=f"kT{g}", tag=f"kT{g}_{rr}")
                       for g in range(N_KV_HEAD)]
                vts = [pact.tile([128, KV_DIM], BF16, name=f"vt{i}",
                                 tag=f"vt{i}_{rr}") for i in range(16)]
                qTt = [pact.tile([128, TQ], BF16, name=f"qT{h}", tag=f"qT{h}_{rr}")
                       for h in range(N_HEAD)]
                yTt = qTt  # y^T reuses the dead roped-q tiles
                kOwn = [pact.tile([128, TQ], BF16, name=f"ko{g}", tag=f"ko{g}_{rr}")
                        for g in range(N_KV_HEAD)]
                vOwn = [pact.tile([128, KV_DIM], BF16, name=f"vo{i}",
                                  tag=f"vo{i}_{rr}") for i in range(8)]

                # rope tables; swap scratch
                ptab = tc.alloc_tile_pool(name="tabs", bufs=1)
                pswk = tc.alloc_tile_pool(name="swk", bufs=2)
                pdram = tc.alloc_tile_pool(name="dramp", bufs=1, space="DRAM")
                kv_in = pdram.tile([TQ, TQ], BF16, name="kv_in", tag=f"kvi{rr}")
                kv_out = pdram.tile([2 * TQ, TQ], BF16, name="kv_out",
                                    tag=f"kvo{rr}")
                # v region views (token-major (1024, 512))
                vin_v = kv_in[512:1024, :].rearrange("a (b c) -> (a b) c", b=2)
                vout_v0 = kv_out[512:1024, :].rearrange("a (b c) -> (a b) c", b=2)
                vout_v1 = kv_out[1536:2048, :].rearrange("a (b c) -> (a b) c", b=2)

                # q rope swap pool (hoisted: lets the h=0 swap DMA prefetch)
                pqs = tc.alloc_tile_pool(name="qsw", bufs=3)
                # rmsnorm stats
                pst = tc.alloc_tile_pool(name="stats", bufs=1)
                ssqk_sb = pst.tile([1, TQ], F32, name="ssqk_sb", tag=f"ssqk{rr}")
                ssqq_sb = pst.tile([1, TQ], F32, name="ssqq_sb", tag=f"ssqq{rr}")
                rs_q = pst.tile([1, TQ], F32, name="rs_q", tag=f"rsq{rr}")
                rs_k = pst.tile([1, TQ], F32, name="rs_k", tag=f"rsk{rr}")
                # x tiles for own tokens (kv proj and q proj)
                pxq = tc.alloc_tile_pool(name="xqp", bufs=1)
                xqs = [pxq.tile([128, TQ], BF16, name=f"xq{kc}", tag=f"xq{kc}_{rr}")
                       for kc in range(16)]

                # ---------------- P1a: k^T and v projections (own half) -------
                pwq = tc.alloc_tile_pool(name="wqlp", bufs=3)
                pwkv = tc.alloc_tile_pool(name="wkv", bufs=1)
                wk4 = [pwkv.tile([128, 4, KV_DIM], BF16, name=f"wk4_{i}",
                                 tag=f"wk4_{i}_{rr}") for i in range(4)]
                wv4 = [pwkv.tile([128, 4, KV_DIM], BF16, name=f"wv4_{i}",
                                 tag=f"wv4_{i}_{rr}") for i in range(4)]
                nc.sync.dma_start(wk4[0][:], rp(wk)[:, 0:4, :])
                for kc in range(4):
                    nc.sync.dma_start(xqs[kc][:], rp(xT)[:, kc, :])
                for i in range(1, 4):
                    nc.sync.dma_start(wk4[i][:], rp(wk)[:, 4 * i:4 * i + 4, :])
                for kc in range(4, 16):
                    nc.sync.dma_start(xqs[kc][:], rp(xT)[:, kc, :])
                for i in range(4):
                    nc.sync.dma_start(wv4[i][:], rp(wv)[:, 4 * i:4 * i + 4, :])
                c2ks = ptab.tile([128, TQ], BF16, name="c2ks", tag=f"c2ks{rr}")
                nc.sync.dma_start(c2ks[:], c2k)
                s2ks = ptab.tile([128, TQ], BF16, name="s2ks", tag=f"s2ks{rr}")
                nc.sync.dma_start(s2ks[:], s2k)
                c2qs = ptab.tile([128, TQ], BF16, name="c2qs", tag=f"c2qs{rr}")
                nc.sync.dma_start(c2qs[:], c2q)
                s2qs = ptab.tile([128, TQ], BF16, name="s2qs", tag=f"s2qs{rr}")
                nc.sync.dma_start(s2qs[:], s2q)

                with tc.tile_pool(name="pp1", bufs=3, space="PSUM") as pp, \
                     tc.tile_pool(name="sq1", bufs=3) as pt, \
                     tc.tile_pool(name="ssqkp", bufs=2, space="PSUM") as pps:
                    for tk in range(2):
                        ssqk_ps = pps.tile([1, 512], F32, name="ssqk", tag="ssqk")
                        for co in range(4):
                            ps = pp.tile([128, 512], F32, name="psk", tag="ps")
                            for kc in range(16):
                                nc.tensor.matmul(
                                    ps[:],
                                    wk4[kc // 4][:, kc % 4, co * 128:(co + 1) * 128],
                                    xqs[kc][:, tk * 512:(tk + 1) * 512],
                                    start=(kc == 0),
                                    stop=(kc == 15),
                                )
                            nc.scalar.activation(
                                kOwn[co][:, tk * 512:(tk + 1) * 512], ps[:],
                                AF.Copy, scale=knw_t[:, co:co + 1])
                            sq = pt.tile([128, 512], BF16, name="sqk", tag="sq")
                            nc.scalar.activation(sq[:], ps[:], AF.Square)
                            nc.tensor.matmul(
                                ssqk_ps[:], onesk_t[:], sq[:],
                                start=(co == 0), stop=(co == 3))
                        nc.scalar.activation(
                            ssqk_sb[:, tk * 512:(tk + 1) * 512], ssqk_ps[:], AF.Copy)
                        for vt in range(4):
                            ps = pp.tile([128, 512], F32, name="psv", tag="ps")
                            for kc in range(16):
                                nc.tensor.matmul(
                                    ps[:],
                                    xqs[kc][:, tk * 512 + vt * 128:
                                            tk * 512 + (vt + 1) * 128],
                                    wv4[kc // 4][:, kc % 4, :],
                                    start=(kc == 0),
                                    stop=(kc == 15),
                                )
                            nc.vector.tensor_copy(vOwn[tk * 4 + vt][:], ps[:])
                pwkv.release()

                # ------- k-side: rmsnorm, rope own half, kv AllGather ----------
                with tc.tile_pool(name="kprep", bufs=1) as pkp:
                    sdk = pkp.tile([1, TQ], F32, name="sdk")
                    nc.scalar.activation(sdk[:], ssqk_sb[:], AF.Sqrt, bias=eps_t[:])
                    nc.vector.reciprocal(rs_k[:], sdk[:])
                    bck = pkp.tile([128, TQ], F32, name="bck")
                    nc.gpsimd.partition_broadcast(bck[:], rs_k[:])
                    nc.vector.tensor_mul(c2ks[:], c2ks[:], bck[:])
                    nc.vector.tensor_mul(s2ks[:], s2ks[:], bck[:])
                    for g in range(N_KV_HEAD):
                        ksw = pswk.tile([128, TQ], BF16, name="ksw", tag="ksw")
                        nc.sync.dma_start(ksw[0:64, :], kOwn[g][64:128, :])
                        nc.sync.dma_start(ksw[64:128, :], kOwn[g][0:64, :])
                        nc.vector.tensor_mul(kOwn[g][:], kOwn[g][:], c2ks[:])
                        nc.vector.tensor_mul(ksw[:], ksw[:], s2ks[:])
                        nc.vector.tensor_add(kOwn[g][:], kOwn[g][:], ksw[:])
                        nc.sync.dma_start(kv_in[g * 128:(g + 1) * 128, :],
                                          kOwn[g][:])
                    for i in range(8):
                        nc.sync.dma_start(vin_v[i * 128:(i + 1) * 128, :],
                                          vOwn[i][:])
                    nc.gpsimd.collective_compute(
                        "AllGather",
                        ALU.bypass,
                        replica_groups=RG,
                        ins=[kv_in.opt()],
                        outs=[kv_out.opt()],
                    )
                    # read back full k (roped) and v in original token order
                    for g in range(N_KV_HEAD):
                        nc.sync.dma_start(kTt[g][:, 0:TQ],
                                          kv_out[g * 128:(g + 1) * 128, :])
                        nc.sync.dma_start(
                            kTt[g][:, TQ:T],
                            kv_out[TQ + g * 128:TQ + (g + 1) * 128, :])
                    for i in range(8):
                        nc.sync.dma_start(vts[i][:],
                                          vout_v0[i * 128:(i + 1) * 128, :])
                        nc.sync.dma_start(vts[8 + i][:],
                                          vout_v1[i * 128:(i + 1) * 128, :])

                # ---------------- P1b: q^T projection --------
                pw = pwq
                with tc.tile_pool(name="sq2", bufs=3) as pt, \
                     tc.tile_pool(name="pp2", bufs=3, space="PSUM") as pp, \
                     tc.tile_pool(name="ssqq0", bufs=1, space="PSUM") as pps0, \
                     tc.tile_pool(name="ssqq1", bufs=1, space="PSUM") as pps1:
                    ssq_ps = [pps0.tile([1, 512], F32, name="ssqq0", tag="ssqq0"),
                              pps1.tile([1, 512], F32, name="ssqq1", tag="ssqq1")]
                    for h in range(16):
                        wql = pw.tile([128, 16, 128], BF16, name="wql", tag="wql")
                        nc.sync.dma_start(wql[:], rp(wq)[:, :, h * 128:(h + 1) * 128])
                        for tq in range(2):
                            ps = pp.tile([128, 512], F32, name="psq", tag="ps")
                            for kc in range(16):
                                nc.tensor.matmul(
                                    ps[:],
                                    wql[:, kc, :],
                                    xqs[kc][:, tq * 512:(tq + 1) * 512],
                                    start=(kc == 0),
                                    stop=(kc == 15),
                                )
                            nc.scalar.activation(
                                qTt[h][:, tq * 512:(tq + 1) * 512], ps[:],
                                AF.Copy, scale=qnw_t[:, h:h + 1])
                            sq = pt.tile([128, 512], BF16, name="sqq", tag="sq")
                            nc.scalar.activation(sq[:], ps[:], AF.Square)
                            nc.tensor.matmul(
                                ssq_ps[tq][:], onesq_t[:], sq[:],
                                start=(h == 0), stop=(h == 15))
                        # rope q with raw tables now (rmsnorm scale applied
                        # per-head at attention time); DVE is idle here
                        qsw = pqs.tile([128, TQ], BF16, name="qsw", tag="qsw")
                        nc.sync.dma_start(qsw[0:64, :], qTt[h][64:128, :])
                        nc.sync.dma_start(qsw[64:128, :], qTt[h][0:64, :])
                        nc.vector.tensor_mul(qTt[h][:], qTt[h][:], c2qs[:])
                        nc.vector.tensor_mul(qsw[:], qsw[:], s2qs[:])
                        nc.vector.tensor_add(qTt[h][:], qTt[h][:], qsw[:])
                    for tq in range(2):
                        nc.scalar.activation(
                            ssqq_sb[:, tq * 512:(tq + 1) * 512], ssq_ps[tq][:],
                            AF.Copy)
                pwq.release()
                pxq.release()

                # ------------- q-side rmsnorm scale (applied per head) --------
                bcq = ptab.tile([128, TQ], F32, name="bcq", tag=f"bcq{rr}")
                with tc.tile_pool(name="bcp", bufs=1) as pbc:
                    sdq = pbc.tile([1, TQ], F32, name="sdq")
                    nc.scalar.activation(sdq[:], ssqq_sb[:], AF.Sqrt, bias=eps_t[:])
                    # preload the exp table set while the DVE/Pool chain runs,
                    # so attention's first real Exp pays no ~2.7us table switch
                    warm = pbc.tile([1, 1], F32, name="warm")
                    nc.scalar.activation(warm[:], eps_t[:], AF.Exp)
                    nc.vector.reciprocal(rs_q[:], sdq[:])
                    nc.gpsimd.partition_broadcast(bcq[:], rs_q[:])
                pst.release()

                # ---------------- P2: attention (+ P3 wo prefetch) -------------
                pwo = tc.alloc_tile_pool(name="wop", bufs=2)
                with tc.tile_pool(name="Sp", bufs=2) as pS, \
                     tc.tile_pool(name="dt8", bufs=2) as pd8, \
                     tc.tile_pool(name="dt4", bufs=2) as pd4, \
                     tc.tile_pool(name="yev", bufs=3) as pye, \
                     tc.tile_pool(name="sps", bufs=2, space="PSUM") as ppS, \
                     tc.tile_pool(name="denp", bufs=2, space="PSUM") as ppd, \
                     tc.tile_pool(name="ytp", bufs=2, space="PSUM") as ppy:
                    for h in range(N_HEAD):
                        g = h // 4
                        # apply rmsnorm scale to the pre-roped q
                        nc.vector.tensor_mul(qTt[h][:], qTt[h][:], bcq[:])
                        for qc in range(2):
                            qsl = qTt[h][:, qc * 512:(qc + 1) * 512]
                            S_sb = pS.tile([128, 16, 512], BF16, name="S_sb", tag="S")
                            for j in range(8):
                                sps = ppS.tile([128, 2, 512], F32, name="sps",
                                               tag="sps")
                                for i in range(2):
                                    kc = 2 * j + i
                                    nc.tensor.matmul(
                                        sps[:, i, :],
                                        kTt[g][:, kc * 128:(kc + 1) * 128],
                                        qsl,
                                        start=True,
                                        stop=True,
                                    )
                                nc.scalar.activation(
                                    S_sb[:, 2 * j:2 * j + 2, :], sps[:], AF.Exp)
                            # denominator: bf16 pair-tree 16 -> 4 on DVE
                            t8 = pd8.tile([128, 8, 512], BF16, name="t8", tag="t8")
                            for i in range(8):
                                nc.vector.tensor_add(
                                    t8[:, i, :], S_sb[:, 2 * i, :],
                                    S_sb[:, 2 * i + 1, :])
                            t4 = pd4.tile([128, 4, 512], BF16, name="t4", tag="t4")
                            for i in range(4):
                                nc.vector.tensor_add(
                                    t4[:, i, :], t8[:, 2 * i, :],
                                    t8[:, 2 * i + 1, :])
                            # y^T = v^T S accumulated over 16 token chunks
                            yt_ps = ppy.tile([128, 512], F32, name="ytp", tag="ytp")
                            for kc in range(16):
                                nc.tensor.matmul(
                                    yt_ps[:],
                                    vts[kc][:, g * 128:(g + 1) * 128],
                                    S_sb[:, kc, :],
                                    start=(kc == 0),
                                    stop=(kc == 15),
                                )
                            den_ps = ppd.tile([1, 512], F32, name="den", tag="den")
                            for i in range(4):
                                nc.tensor.matmul(
                                    den_ps[:], ones_t[:], t4[:, i, :],
                                    start=(i == 0), stop=(i == 3))
                            rcp = pye.tile([1, 512], F32, name="rcp", tag="rcp")
                            nc.vector.reciprocal(rcp[:], den_ps[:])
                            bcr = pye.tile([128, 512], F32, name="bcr", tag="bcr")
                            nc.gpsimd.partition_broadcast(bcr[:], rcp[:])
                            nc.vector.tensor_mul(
                                yTt[h][:, qc * 512:(qc + 1) * 512], yt_ps[:], bcr[:])

                # ---------------- P3: output projection ----------------
                with tc.tile_pool(name="ev3", bufs=4) as pe3, \
                     tc.tile_pool(name="pp3", bufs=4, space="PSUM") as pp3:
                    for co in range(4):
                        woc = pwo.tile([128, 16, 512], BF16, name="woc",
                                       tag="woc")
                        for yc in range(16):
                            nc.sync.dma_start(
                                woc[:, yc, :],
                                rp(wo)[:, yc, co * 512:(co + 1) * 512])
                        for qt in range(8):
                            ps = pp3.tile([128, 512], F32, name="pso", tag="ps")
                            for yc in range(16):
                                nc.tensor.matmul(
                                    ps[:],
                                    yTt[yc][:, qt * 128:(qt + 1) * 128],
                                    woc[:, yc, :],
                                    start=(yc == 0),
                                    stop=(yc == 15),
                                )
                            osb = pe3.tile([128, 512], F32, name="osb",
                                           tag="osb")
                            nc.vector.tensor_copy(osb[:], ps[:])
                            nc.sync.dma_start(
                                out[qt * 128:(qt + 1) * 128,
                                    co * 512:(co + 1) * 512],
                                osb[:],
                            )
                pwo.release()
                pqs.release()
                pdram.release()
                pswk.release()
                ptab.release()
                pact.release()

    nc.compile()
    return nc


def _make_in_maps(inputs):
    x = np.asarray(inputs["x"], np.float32)
    cos = np.asarray(inputs["cos"], np.float32)
    sin = np.asarray(inputs["sin"], np.float32)
    wq = np.ascontiguousarray(np.asarray(inputs["wq"], np.float32)).astype(BF)
    wk = np.ascontiguousarray(np.asarray(inputs["wk"], np.float32)).astype(BF)
    wv = np.ascontiguousarray(np.asarray(inputs["wv"], np.float32)).astype(BF)
    wo = np.ascontiguousarray(np.asarray(inputs["wo"], np.float32)).astype(BF)
    qnw = np.ascontiguousarray(
        np.asarray(inputs["q_norm_w"], np.float32).reshape(16, 128).T)
    knw = np.ascontiguousarray(
        np.asarray(inputs["k_norm_w"], np.float32).reshape(4, 128).T)

    cf = cos[0, :, 0, :].T  # (64, T)
    sf = sin[0, :, 0, :].T
    c2 = np.concatenate([cf, cf], 0)  # (128, T)
    s2 = np.concatenate([sf, -sf], 0)
    scale = 1.0 / np.sqrt(np.float32(HEAD_DIM))

    in_maps = []
    for c in range(N_CORES):
        b, r0 = c // 2, (c % 2) * TQ
        xTb = x[b].T  # (C, T)
        in_maps.append({
            "xT": np.ascontiguousarray(xTb[:, r0:r0 + TQ]).astype(BF),
            "wq": wq, "wk": wk, "wv": wv, "wo": wo,
            "c2q": np.ascontiguousarray(c2[:, r0:r0 + TQ] * scale).astype(BF),
            "s2q": np.ascontiguousarray(s2[:, r0:r0 + TQ] * scale).astype(BF),
            "c2k": np.ascontiguousarray(c2[:, r0:r0 + TQ]).astype(BF),
            "s2k": np.ascontiguousarray(s2[:, r0:r0 + TQ]).astype(BF),
            "qnw": qnw, "knw": knw,
        })
    return in_maps


def run(inputs, **spmd_kwargs):
    from concourse import bass_utils

    if "nc" not in _CACHE:
        _CACHE["nc"] = _build_nc()
    nc = _CACHE["nc"]
    res = bass_utils.run_bass_kernel_spmd(
        nc, _make_in_maps(inputs), core_ids=list(range(N_CORES)), **spmd_kwargs
    )
    out = np.empty((B, T, C), np.float32)
    for c in range(N_CORES):
        b, r0 = c // 2, (c % 2) * TQ
        out[b, r0:r0 + TQ, :] = res.results[c]["out"]
    return out, res


def kernel(**inputs):
    out, _ = run(inputs)
    return out
